# revision 4
# baseline (speedup 1.0000x reference)
"""Single-level 2D Haar DWT (periodization mode) on Trainium2.

Input x: (8, 512, 512, 16) fp32 NHWC. Output: (LL, LH, HL, HH), each
(8, 256, 256, 16) fp32 — +/- combinations of each 2x2 spatial block,
scaled by 0.5.

Sharding: pure data parallel — one batch sample per NeuronCore (8 cores).

All device I/O is fp16 (host casts; the x0.5 subband scale is applied
during the host-side fp16 -> fp32 upcast), so per-core traffic is
8.4 MB in + 8.4 MB out. The DMA fabric is ~425 GB/s aggregate (16
engines x ~25 GB/s shared by all queues), but a single queue only
sustains ~290 GB/s, so both the input and output streams are split
across two queues each to keep the fabric dense:

  Q0 (GpSimd SWDGE): B-path loads (4.2 MB) + B LH/HH stores (2.1 MB)
  Q1 (SP HWDGE):     weight + merged A stores (4.2 MB) + B LL/HL
                     stores (2.1 MB)
  Q10 (ACT HWDGE):   A-path loads (4.2 MB)

Loads always precede stores in each queue's FIFO, so chains never
self-block (A: load Q10 -> compute -> store Q1/Q0; B: load Q0 ->
DVE -> store Q1/Q0-after-loads).

Work is split by W-halves across two compute paths:

Path A (W cols 0:4096) — TensorE + ScalarE + VectorE, 8 units of
  128 rows x 2048 cols (512 KB):
  - TensorE: row-direction (H) butterfly as matmul with a fixed
    128x128 +/-1 fp16 weight (PSUM rows 0..63 = top+bot, 64..127 =
    top-bot); 4 matmuls of 512 cols per unit.
  - ScalarE (ACT): PSUM -> SBUF copy, fp32 -> fp16.
  - VectorE: column (W) butterfly, even +/- odd, writing into
    per-kc [128, 2048] sum/diff tiles (g=0 left half, g=1 right).
  - After both g's: 4 merged stores of [64 rows, 4 KB] per kc.

Path B (W cols 4096:8192) — VectorE only, 2 units of 128 row-pairs x
  4096 cols (2 MB): classic 8-op butterfly.

Schedule: every tile is resident in SBUF (no pool-buffer reuse,
~195 KB/partition); all input loads issue right after the preamble
on two parallel queues, so input lands by ~20 us instead of 40.
Emission order sets per-engine priorities: B0's top tile loads first
so VectorE starts at ~11 us; A units follow in land order; B-path
ops fill VectorE gaps.

Measured constraints this layout respects:
  - store descriptors >= 2 KB contiguous (smaller ran ~10% slow);
  - DMA dst APs keep a large outermost dim (engine-spread is over
    the outermost dst dim);
  - each dma_start costs ~650 ns issue time on its engine (HWDGE)
    or ~1 us (SWDGE), and issue instructions stall when the queue
    backs up — hence few, large stores;
  - Bacc built with num_devices=1 (no collectives needed).
"""

import sys

if "/opt/trn_rl_repo" not in sys.path:
    sys.path.insert(0, "/opt/trn_rl_repo")

import numpy as np

B, H, W, C = 8, 512, 512, 16
N_CORES = 8
HO, WO = H // 2, W // 2  # 256, 256
ROW = W * C  # 8192 elements per input row
OROW = WO * C  # 4096 elements per output row

_CACHE = {}


def _haar_weight():
    """lhsT [k, m]: matmul computes out[m, n] = sum_k w[k, m] x[k, n]."""
    w = np.zeros((128, 128), dtype=np.float16)
    for m in range(64):
        w[2 * m, m] = 1.0
        w[2 * m + 1, m] = 1.0
        w[2 * m, 64 + m] = 1.0
        w[2 * m + 1, 64 + m] = -1.0
    return w


def _build():
    import concourse.bacc as bacc
    import concourse.mybir as mybir
    import concourse.tile as tile

    fp32 = mybir.dt.float32
    fp16 = mybir.dt.float16

    nc = bacc.Bacc(
        "TRN2", target_bir_lowering=False, debug=False, num_devices=1
    )
    x = nc.dram_tensor("x", (H, ROW), fp16, kind="ExternalInput")
    wdram = nc.dram_tensor("w", (128, 128), fp16, kind="ExternalInput")
    outs = {
        name: nc.dram_tensor(name, (HO, OROW), fp16, kind="ExternalOutput")
        for name in ("LL", "LH", "HL", "HH")
    }

    xq = x.rearrange("(q t) m -> q t m", t=2)  # [pair, row-parity, cols]

    HALF = ROW // 2  # 4096: A path covers cols 0:HALF, B path HALF:ROW
    AW = 2048  # A unit width (input cols); 4 matmuls of 512
    MM_N = 512  # one fp32 matmul / PSUM bank
    A_UNITS = [(kc, g) for kc in range(4) for g in range(2)]

    with tile.TileContext(nc) as tc:
        with (
            tc.tile_pool(name="main", bufs=1) as pool,
            tc.tile_pool(name="psum", bufs=2, space="PSUM") as psum,
        ):
            wt = pool.tile([128, 128], fp16, tag="wt")
            nc.sync.dma_start(wt[:], wdram[:])

            # ---- all input loads, two parallel queues ----
            tops = {}
            bots = {}
            for pc in range(2):
                tops[pc] = pool.tile(
                    [128, HALF], fp16, tag=f"top{pc}", name=f"top{pc}"
                )
                bots[pc] = pool.tile(
                    [128, HALF], fp16, tag=f"bot{pc}", name=f"bot{pc}"
                )
            xts = {}
            for kc, g in A_UNITS:
                xts[(kc, g)] = pool.tile(
                    [128, AW], fp16, tag=f"xt{kc}{g}", name=f"xt{kc}{g}"
                )

            # B loads on Q0 (SWDGE): B0.top first so DVE starts early.
            for pc in range(2):
                qs = slice(pc * 128, (pc + 1) * 128)
                nc.gpsimd.dma_start(tops[pc][:], xq[qs, 0, HALF:ROW])
                nc.gpsimd.dma_start(bots[pc][:], xq[qs, 1, HALF:ROW])
            # A loads on Q10 (ACT HWDGE), consumption order.
            for kc, g in A_UNITS:
                nc.scalar.dma_start(
                    xts[(kc, g)][:],
                    x[kc * 128 : (kc + 1) * 128, g * AW : (g + 1) * AW],
                )

            # ---- B0 mids: highest DVE priority (only ready work early) ----
            mids = {}
            for pc in range(2):
                for mt in ("t1", "t2", "u1", "u2"):
                    mids[(pc, mt)] = pool.tile(
                        [128, HALF // 2],
                        fp16,
                        tag=f"m{mt}{pc}",
                        name=f"m{mt}{pc}",
                    )

            def emit_b_mids(pc):
                tv = tops[pc][:].rearrange("p (w u c) -> p w u c", u=2, c=C)
                bv = bots[pc][:].rearrange("p (w u c) -> p w u c", u=2, c=C)
                a, b = tv[:, :, 0, :], tv[:, :, 1, :]
                c_, d = bv[:, :, 0, :], bv[:, :, 1, :]
                m = lambda mt: mids[(pc, mt)][:].rearrange(
                    "p (w c) -> p w c", c=C
                )
                # top-only ops first: they unblock as soon as `top` lands
                nc.vector.tensor_add(m("t1"), a, b)
                nc.vector.tensor_sub(m("u1"), a, b)
                nc.vector.tensor_add(m("t2"), c_, d)
                nc.vector.tensor_sub(m("u2"), c_, d)

            emit_b_mids(0)

            # ---- A units in land order ----
            # sums[kc]/diffs[kc] are [128, 2048]: g=0 writes cols 0:1024,
            # g=1 cols 1024:2048, so each kc needs only 4 merged stores.
            sums = {}
            diffs = {}
            for kc in range(4):
                sums[kc] = pool.tile(
                    [128, AW], fp16, tag=f"s{kc}", name=f"s{kc}"
                )
                diffs[kc] = pool.tile(
                    [128, AW], fp16, tag=f"d{kc}", name=f"d{kc}"
                )

            def emit_a_unit(kc, g):
                xt = xts[(kc, g)]
                ps = psum.tile([128, AW], fp32)
                for j in range(AW // MM_N):
                    lo = j * MM_N
                    nc.tensor.matmul(
                        ps[:, lo : lo + MM_N],
                        wt[:],
                        xt[:, lo : lo + MM_N],
                        start=True,
                        stop=True,
                    )
                sb = pool.tile([128, AW], fp16, tag=f"sb{kc}{g}")
                nc.scalar.copy(sb[:], ps[:])  # ACT: PSUM -> SBUF, fp32->fp16
                sv_in = sb[:].rearrange("p (w u c) -> p w u c", u=2, c=C)
                ev, od = sv_in[:, :, 0, :], sv_in[:, :, 1, :]
                half = slice(g * (AW // 2), (g + 1) * (AW // 2))
                sv = sums[kc][:, half].rearrange("p (w c) -> p w c", c=C)
                dv = diffs[kc][:, half].rearrange("p (w c) -> p w c", c=C)
                nc.vector.tensor_add(sv, ev, od)
                nc.vector.tensor_sub(dv, ev, od)

            def emit_a_stores(kc):
                rs = slice(kc * 64, (kc + 1) * 64)
                cols = slice(0, AW)
                nc.sync.dma_start(outs["LL"][rs, cols], sums[kc][0:64, :])
                nc.sync.dma_start(outs["HL"][rs, cols], sums[kc][64:128, :])
                nc.sync.dma_start(outs["LH"][rs, cols], diffs[kc][0:64, :])
                nc.sync.dma_start(outs["HH"][rs, cols], diffs[kc][64:128, :])

            for kc in range(4):
                emit_a_unit(kc, 0)
                emit_a_unit(kc, 1)
                emit_a_stores(kc)

            # ---- B outs: fill DVE gaps; stores split Q1/Q0 ----
            def emit_b_outs(pc):
                qs = slice(pc * 128, (pc + 1) * 128)
                oc = slice(HALF // 2, OROW)
                WQ = HALF // (2 * C)
                for name, i0, i1, op, ring in (
                    ("LL", "t1", "t2", "add", "sync"),
                    ("HL", "t1", "t2", "sub", "sync"),
                    ("LH", "u1", "u2", "add", "gpsimd"),
                    ("HH", "u1", "u2", "sub", "gpsimd"),
                ):
                    ot = pool.tile([128, WQ, C], fp16, tag=f"o{name}{pc}")
                    a0 = mids[(pc, i0)][:].rearrange("p (w c) -> p w c", c=C)
                    a1 = mids[(pc, i1)][:].rearrange("p (w c) -> p w c", c=C)
                    if op == "add":
                        nc.vector.tensor_add(ot[:], a0, a1)
                    else:
                        nc.vector.tensor_sub(ot[:], a0, a1)
                    eng = nc.sync if ring == "sync" else nc.gpsimd
                    eng.dma_start(
                        outs[name][qs, oc],
                        ot[:].rearrange("p w c -> p (w c)"),
                    )

            emit_b_outs(0)
            emit_b_mids(1)
            emit_b_outs(1)

    nc.compile()
    return nc


def _get_nc():
    if "nc" not in _CACHE:
        _CACHE["nc"] = _build()
    return _CACHE["nc"]


def _in_maps(x):
    w = _haar_weight()
    xh = np.asarray(x, dtype=np.float16)
    return [
        {"x": np.ascontiguousarray(xh[i].reshape(H, ROW)), "w": w}
        for i in range(B)
    ]


def kernel(x):
    from concourse.bass_utils import run_bass_kernel_spmd

    x = np.asarray(x, dtype=np.float32)
    assert x.shape == (B, H, W, C), x.shape

    nc = _get_nc()
    try:
        res = run_bass_kernel_spmd(nc, _in_maps(x), list(range(N_CORES)))
    except Exception:
        # transient NRT device errors have been observed right after
        # compile; one retry has always succeeded
        res = run_bass_kernel_spmd(nc, _in_maps(x), list(range(N_CORES)))

    out = []
    for name in ("LL", "LH", "HL", "HH"):
        sub = np.stack(
            [res.results[i][name].reshape(HO, WO, C) for i in range(B)],
            axis=0,
        )
        out.append(sub.astype(np.float32) * np.float32(0.5))
    return tuple(out)


# revision 5
# speedup vs baseline: 1.0657x; 1.0657x over previous
"""Single-level 2D Haar DWT (periodization mode) on Trainium2.

Input x: (8, 512, 512, 16) fp32 NHWC. Output: (LL, LH, HL, HH), each
(8, 256, 256, 16) fp32 — +/- combinations of each 2x2 spatial block,
scaled by 0.5.

Sharding: pure data parallel — one batch sample per NeuronCore (8 cores).

All device I/O is fp16 (host casts; the x0.5 subband scale is applied
during the host-side fp16 -> fp32 upcast), so per-core traffic is
8.4 MB in + 8.4 MB out.

DMA model measured on this part (SPMD, all 8 cores streaming):
  - fabric total ~425 GB/s per core (chip HBM share);
  - one SWDGE queue (GpSimd) sustains ~290-330 GB/s (16 engines per
    instruction); one HWDGE queue (SP or ACT ring) only ~170-210
    (descriptors of one instruction spread over ~8 engines);
  - at most ~8 SWDGE dma_starts can be outstanding before the Tile
    framework inserts a GpSimd DRAIN (descriptor scratch recycle),
    which stalls later issues — so Q0 carries exactly 8;
  - each dma_start costs ~650 ns (HWDGE) / ~1 us (SWDGE) of issue
    time on its engine, and issue instructions stall when the queue
    backs up — hence few, large transfers (>= 2 KB descriptors).

Queue plan (bytes balanced to per-queue ceilings):
  Q0  (GpSimd SWDGE): B loads + A kc=2,3 loads (6.3 MB) then
                      B0 LH/HH stores (1.05 MB) — 8 instructions
  Q1  (SP HWDGE):     weight + A kc=1 loads (1 MB) + 16 merged A
                      stores (4.2 MB)
  Q10 (ACT HWDGE):    A kc=0 loads (1 MB) + B0 LL/HL + all B1
                      stores (3.15 MB)
The kc=0/1 loads ride the HWDGE queues' otherwise idle early window,
so the A pipeline starts at ~10 us and Q0 is 2 MB lighter.

Work split by W-halves across two compute paths:

Path A (W cols 0:4096) — TensorE + ScalarE + VectorE, 8 units of
  128 rows x 2048 cols (512 KB):
  - TensorE: row (H) butterfly as matmul with a fixed 128x128 +/-1
    fp16 weight (PSUM rows 0..63 = top+bot, 64..127 = top-bot).
  - ScalarE (ACT): PSUM -> SBUF copy, fp32 -> fp16.
  - VectorE: column (W) butterfly into per-kc [128, 2048] sum/diff
    tiles (g=0 left half, g=1 right) -> 4 merged [64 row, 4 KB]
    stores per kc.

Path B (W cols 4096:8192) — VectorE only, 2 units of 128 row-pairs x
  4096 cols (2 MB): classic 8-op butterfly.

Schedule: every tile is resident in SBUF (~193 KB/partition, no
pool-buffer reuse); all loads issue right after the preamble across
three parallel queues. Emission order sets per-engine priorities:
B0.top loads first so VectorE starts ~11 us; A units in land order;
B outs fill VectorE gaps; A kc=2,3 land last because the A chain has
the shortest post-land latency (best tail).

Bacc is built with num_devices=1: no collectives needed.
"""

import sys

if "/opt/trn_rl_repo" not in sys.path:
    sys.path.insert(0, "/opt/trn_rl_repo")

import numpy as np

B, H, W, C = 8, 512, 512, 16
N_CORES = 8
HO, WO = H // 2, W // 2  # 256, 256
ROW = W * C  # 8192 elements per input row
OROW = WO * C  # 4096 elements per output row

_CACHE = {}


def _haar_weight():
    """lhsT [k, m]: matmul computes out[m, n] = sum_k w[k, m] x[k, n]."""
    w = np.zeros((128, 128), dtype=np.float16)
    for m in range(64):
        w[2 * m, m] = 1.0
        w[2 * m + 1, m] = 1.0
        w[2 * m, 64 + m] = 1.0
        w[2 * m + 1, 64 + m] = -1.0
    return w


def _build():
    import concourse.bacc as bacc
    import concourse.mybir as mybir
    import concourse.tile as tile

    fp32 = mybir.dt.float32
    fp16 = mybir.dt.float16

    nc = bacc.Bacc(
        "TRN2", target_bir_lowering=False, debug=False, num_devices=1
    )
    x = nc.dram_tensor("x", (H, ROW), fp16, kind="ExternalInput")
    wdram = nc.dram_tensor("w", (128, 128), fp16, kind="ExternalInput")
    outs = {
        name: nc.dram_tensor(name, (HO, OROW), fp16, kind="ExternalOutput")
        for name in ("LL", "LH", "HL", "HH")
    }

    xq = x.rearrange("(q t) m -> q t m", t=2)  # [pair, row-parity, cols]

    HALF = ROW // 2  # 4096: A path covers cols 0:HALF, B path HALF:ROW
    AW = 2048  # A unit width (input cols); 4 matmuls of 512
    MM_N = 512  # one fp32 matmul / PSUM bank

    with tile.TileContext(nc) as tc:
        with (
            tc.tile_pool(name="main", bufs=1) as pool,
            tc.tile_pool(name="psum", bufs=2, space="PSUM") as psum,
        ):
            wt = pool.tile([128, 128], fp16, tag="wt")
            nc.sync.dma_start(wt[:], wdram[:])

            # ---- tiles ----
            tops = {}
            bots = {}
            for pc in range(2):
                tops[pc] = pool.tile(
                    [128, HALF], fp16, tag=f"top{pc}", name=f"top{pc}"
                )
                bots[pc] = pool.tile(
                    [128, HALF], fp16, tag=f"bot{pc}", name=f"bot{pc}"
                )
            # kc=0,1: two half tiles each (separate loads -> earlier MM
            # start); kc=2,3: one [128, 4096] tile, one SWDGE load each.
            xth = {}
            for kc in range(2):
                for g in range(2):
                    xth[(kc, g)] = pool.tile(
                        [128, AW], fp16, tag=f"xt{kc}{g}", name=f"xt{kc}{g}"
                    )
            xtf = {}
            for kc in range(2, 4):
                xtf[kc] = pool.tile(
                    [128, HALF], fp16, tag=f"xt{kc}", name=f"xt{kc}"
                )

            def a_src(kc, g):
                if kc < 2:
                    return xth[(kc, g)][:]
                return xtf[kc][:, g * AW : (g + 1) * AW]

            # ---- loads ----
            # Q10 (scalar): kc=0 halves — land ~10/12.5 us.
            for g in range(2):
                nc.scalar.dma_start(
                    xth[(0, g)][:], x[0:128, g * AW : (g + 1) * AW]
                )
            # Q1 (sync): kc=1 halves.
            for g in range(2):
                nc.sync.dma_start(
                    xth[(1, g)][:], x[128:256, g * AW : (g + 1) * AW]
                )
            # Q0 (gpsimd): B0, B1, then A kc=2,3 (shortest tail chain last).
            for pc in range(2):
                qs = slice(pc * 128, (pc + 1) * 128)
                nc.gpsimd.dma_start(tops[pc][:], xq[qs, 0, HALF:ROW])
                nc.gpsimd.dma_start(bots[pc][:], xq[qs, 1, HALF:ROW])
            for kc in range(2, 4):
                nc.gpsimd.dma_start(
                    xtf[kc][:], x[kc * 128 : (kc + 1) * 128, 0:HALF]
                )

            # ---- B0 mids: highest DVE priority (only ready work early) ----
            mids = {}
            for pc in range(2):
                for mt in ("t1", "t2", "u1", "u2"):
                    mids[(pc, mt)] = pool.tile(
                        [128, HALF // 2],
                        fp16,
                        tag=f"m{mt}{pc}",
                        name=f"m{mt}{pc}",
                    )

            def emit_b_mids(pc):
                tv = tops[pc][:].rearrange("p (w u c) -> p w u c", u=2, c=C)
                bv = bots[pc][:].rearrange("p (w u c) -> p w u c", u=2, c=C)
                a, b = tv[:, :, 0, :], tv[:, :, 1, :]
                c_, d = bv[:, :, 0, :], bv[:, :, 1, :]
                m = lambda mt: mids[(pc, mt)][:].rearrange(
                    "p (w c) -> p w c", c=C
                )
                # top-only ops first: they unblock as soon as `top` lands
                nc.vector.tensor_add(m("t1"), a, b)
                nc.vector.tensor_sub(m("u1"), a, b)
                nc.vector.tensor_add(m("t2"), c_, d)
                nc.vector.tensor_sub(m("u2"), c_, d)

            emit_b_mids(0)

            # ---- A units in land order ----
            sums = {}
            diffs = {}
            for kc in range(4):
                sums[kc] = pool.tile(
                    [128, AW], fp16, tag=f"s{kc}", name=f"s{kc}"
                )
                diffs[kc] = pool.tile(
                    [128, AW], fp16, tag=f"d{kc}", name=f"d{kc}"
                )

            def emit_a_unit(kc, g):
                xt = a_src(kc, g)
                ps = psum.tile([128, AW], fp32)
                for j in range(AW // MM_N):
                    lo = j * MM_N
                    nc.tensor.matmul(
                        ps[:, lo : lo + MM_N],
                        wt[:],
                        xt[:, lo : lo + MM_N],
                        start=True,
                        stop=True,
                    )
                sb = pool.tile([128, AW], fp16, tag=f"sb{kc}{g}")
                nc.scalar.copy(sb[:], ps[:])  # ACT: PSUM -> SBUF, fp32->fp16
                sv_in = sb[:].rearrange("p (w u c) -> p w u c", u=2, c=C)
                ev, od = sv_in[:, :, 0, :], sv_in[:, :, 1, :]
                half = slice(g * (AW // 2), (g + 1) * (AW // 2))
                sv = sums[kc][:, half].rearrange("p (w c) -> p w c", c=C)
                dv = diffs[kc][:, half].rearrange("p (w c) -> p w c", c=C)
                nc.vector.tensor_add(sv, ev, od)
                nc.vector.tensor_sub(dv, ev, od)

            def emit_a_stores(kc):
                rs = slice(kc * 64, (kc + 1) * 64)
                cols = slice(0, AW)
                nc.sync.dma_start(outs["LL"][rs, cols], sums[kc][0:64, :])
                nc.sync.dma_start(outs["HL"][rs, cols], sums[kc][64:128, :])
                nc.sync.dma_start(outs["LH"][rs, cols], diffs[kc][0:64, :])
                nc.sync.dma_start(outs["HH"][rs, cols], diffs[kc][64:128, :])

            for kc in range(4):
                emit_a_unit(kc, 0)
                emit_a_unit(kc, 1)
                emit_a_stores(kc)

            # ---- B outs: fill DVE gaps ----
            def emit_b_outs(pc, rings):
                qs = slice(pc * 128, (pc + 1) * 128)
                oc = slice(HALF // 2, OROW)
                WQ = HALF // (2 * C)
                for (name, i0, i1, op), ring in zip(
                    (
                        ("LL", "t1", "t2", "add"),
                        ("HL", "t1", "t2", "sub"),
                        ("LH", "u1", "u2", "add"),
                        ("HH", "u1", "u2", "sub"),
                    ),
                    rings,
                ):
                    ot = pool.tile([128, WQ, C], fp16, tag=f"o{name}{pc}")
                    a0 = mids[(pc, i0)][:].rearrange("p (w c) -> p w c", c=C)
                    a1 = mids[(pc, i1)][:].rearrange("p (w c) -> p w c", c=C)
                    if op == "add":
                        nc.vector.tensor_add(ot[:], a0, a1)
                    else:
                        nc.vector.tensor_sub(ot[:], a0, a1)
                    ring.dma_start(
                        outs[name][qs, oc],
                        ot[:].rearrange("p w c -> p (w c)"),
                    )

            emit_b_outs(0, (nc.scalar, nc.scalar, nc.gpsimd, nc.gpsimd))
            emit_b_mids(1)
            emit_b_outs(1, (nc.scalar, nc.scalar, nc.scalar, nc.scalar))

    nc.compile()
    return nc


def _get_nc():
    if "nc" not in _CACHE:
        _CACHE["nc"] = _build()
    return _CACHE["nc"]


def _in_maps(x):
    w = _haar_weight()
    xh = np.asarray(x, dtype=np.float16)
    return [
        {"x": np.ascontiguousarray(xh[i].reshape(H, ROW)), "w": w}
        for i in range(B)
    ]


def kernel(x):
    from concourse.bass_utils import run_bass_kernel_spmd

    x = np.asarray(x, dtype=np.float32)
    assert x.shape == (B, H, W, C), x.shape

    nc = _get_nc()
    try:
        res = run_bass_kernel_spmd(nc, _in_maps(x), list(range(N_CORES)))
    except Exception:
        # transient NRT device errors have been observed right after
        # compile; one retry has always succeeded
        res = run_bass_kernel_spmd(nc, _in_maps(x), list(range(N_CORES)))

    out = []
    for name in ("LL", "LH", "HL", "HH"):
        sub = np.stack(
            [res.results[i][name].reshape(HO, WO, C) for i in range(B)],
            axis=0,
        )
        out.append(sub.astype(np.float32) * np.float32(0.5))
    return tuple(out)
